# revision 2
# baseline (speedup 1.0000x reference)
"""Trainium2 Bass kernel for nn_Compressor (sparse_attention, hierarchical window MLP).

Reference computation (per batch b, head h):
  windows w=0..510 over k[b,h] (S=8192, D=128), window length 32, stride 16
  x[w, l, :] = k[16w+l, :] + pe[l, :]
  5 stages of pairwise-merge MLP: x <- silu(x.reshape(-1, 256) @ w_down[i].T)
  out[w+1] = x @ w_stop.T   ; out[0] = 0 (prepended zero window)

Sharding: head-parallel across 8 cores (B*H = 32 -> 4 heads/core), weights
replicated, no cross-device comms.

Stage-0 sharing: adjacent row pairs (2t, 2t+1) are shared by two windows in
the same even/odd role, so Z[:, t] = W0_even @ kT[:, 2t] + W0_odd @ kT[:, 2t+1]
is computed once per pair; the window-position-dependent pe enters through the
ScalarE activation bias: s0[:, j, w] = silu(Z[:, 8w+j'] + W0 @ pe_pair_j).

This version processes the core's 4 heads in lockstep so that every ScalarE
activation instruction covers 4*511 = 2044 columns (ACT is the bottleneck
engine: every silu element costs 1 ACT cycle at 1.2 GHz, and each instruction
has a ~90-cycle overhead).  PSUM is split into two 4-bank quads (one bank per
head) that ping-pong between PE writes and ACT reads.  k is fully transposed
on the host into per-e-plane chunks so device DMAs are plain contiguous
transfers (no xbar DMA-transpose).
"""

import numpy as np

B, H, S, D = 2, 16, 8192, 128
BH = B * H
NCORES = 8
HPC = BH // NCORES  # heads per core = 4
NB = (S - 32) // 16 + 1  # 511 sliding windows
NW = NB + 1  # 512 output rows per head (incl. zero window)

_BASS_CACHE = {}


def _build_bass():
    import concourse.bacc as bacc
    import concourse.mybir as mybir
    import concourse.tile as tile

    f32 = mybir.dt.float32
    bf16 = mybir.dt.bfloat16
    SILU = mybir.ActivationFunctionType.Silu

    nc = bacc.Bacc()
    # k5[e, d, h, parity, c] = k[head h, row 16c + 2e + parity, d]  (bf16)
    k5 = nc.dram_tensor("k5", [8, 128, HPC, 2, 512], bf16, kind="ExternalInput")
    wdt = nc.dram_tensor("wdt", [5, 2, 128, 128], bf16, kind="ExternalInput")
    pe0 = nc.dram_tensor("pe0", [128, 16], f32, kind="ExternalInput")
    wst = nc.dram_tensor("wst", [128, 128], bf16, kind="ExternalInput")
    # out_d[o, h, w] = out[head h, window w+1, feature o]  (bf16, host transposes)
    out_d = nc.dram_tensor("out_d", [128, HPC, NB], bf16, kind="ExternalOutput")

    with tile.TileContext(nc) as tc:
        with (
            tc.tile_pool(name="consts", bufs=1) as consts,
            tc.tile_pool(name="kqp", bufs=3) as kqp,
            tc.tile_pool(name="sbp", bufs=1) as sbp,
            tc.tile_pool(name="psA", bufs=1, space="PSUM") as psA,
            tc.tile_pool(name="psB", bufs=1, space="PSUM") as psB,
        ):
            wd_sb = consts.tile([128, 5, 2, 128], bf16, name="wd_sb")
            nc.gpsimd.dma_start(out=wd_sb, in_=wdt.rearrange("i h k o -> k i h o"))
            pe0_sb = consts.tile([128, 16], f32, name="pe0_sb")
            nc.gpsimd.dma_start(out=pe0_sb, in_=pe0[:])
            wst_sb = consts.tile([128, 128], bf16, name="wst_sb")
            nc.gpsimd.dma_start(out=wst_sb, in_=wst[:])

            # two 4-bank PSUM quads (bank = one head)
            zq = psA.tile([128, HPC, 512], f32, name="zq")
            sp = psB.tile([128, HPC, 512], f32, name="sp")

            # SBUF stage outputs, all [128, planes, heads, windows] bf16
            s0 = sbp.tile([128, 16, HPC, NB], bf16, name="s0")
            s1 = sbp.tile([128, 8, HPC, NB], bf16, name="s1")
            s2 = sbp.tile([128, 4, HPC, NB], bf16, name="s2")
            s3 = sbp.tile([128, 2, HPC, NB], bf16, name="s3")
            s4 = sbp.tile([128, HPC, NB], bf16, name="s4")
            out_sb = sbp.tile([128, HPC, NB], bf16, name="out_sb")

            kts = {}

            def dma_e(e):
                kq = kqp.tile([128, HPC, 2, 512], bf16, name="kq")
                nc.sync.dma_start(out=kq, in_=k5[e])
                kts[e] = kq

            def z_unit(e):
                kq = kts.pop(e)
                for par in range(2):
                    for h in range(HPC):
                        nc.tensor.matmul(
                            zq[:, h, :], lhsT=wd_sb[:, 0, par, :],
                            rhs=kq[:, h, par, :],
                            start=(par == 0), stop=(par == 1),
                        )
                nc.scalar.activation(
                    out=s0[:, e, :, :], in_=zq[:, :, 0:NB], func=SILU,
                    bias=pe0_sb[:, e : e + 1], scale=1.0,
                )
                nc.scalar.activation(
                    out=s0[:, e + 8, :, :], in_=zq[:, :, 1 : NB + 1], func=SILU,
                    bias=pe0_sb[:, e + 8 : e + 9], scale=1.0,
                )

            def stage_unit(st, p, prev, cur, region):
                for par in range(2):
                    for h in range(HPC):
                        nc.tensor.matmul(
                            region[:, h, 0:NB], lhsT=wd_sb[:, st, par, :],
                            rhs=prev[:, 2 * p + par, h, :],
                            start=(par == 0), stop=(par == 1),
                        )
                nc.scalar.activation(
                    out=cur[:, p, :, :], in_=region[:, :, 0:NB], func=SILU,
                )

            # prefetch first k chunks
            for e in range(3):
                dma_e(e)

            # phase A: Z(e) interleaved with ready stage-1 planes; the two
            # PSUM quads alternate so PE always has a free region to write
            s1_ready = []  # stage-1 planes ready to emit
            for e in range(8):
                z_unit(e)
                if e + 3 < 8:
                    dma_e(e + 3)
                if e % 2 == 1:
                    k = (e - 1) // 2
                    s1_ready += [k, k + 4]
                if s1_ready and e >= 1:
                    p = s1_ready.pop(0)
                    stage_unit(1, p, s0, s1, sp)
            # leftover stage-1 planes
            for p in s1_ready:
                stage_unit(1, p, s0, s1, sp)

            # phase B: stages 2..4, alternating regions
            for q in range(4):
                stage_unit(2, q, s1, s2, zq if q % 2 == 0 else sp)
            for r in range(2):
                stage_unit(3, r, s2, s3, zq if r % 2 == 0 else sp)

            # stage 4 into zq, per-head activations so the stop pipeline
            # can drain head-by-head
            for par in range(2):
                for h in range(HPC):
                    nc.tensor.matmul(
                        zq[:, h, 0:NB], lhsT=wd_sb[:, 4, par, :],
                        rhs=s3[:, par, h, :],
                        start=(par == 0), stop=(par == 1),
                    )
            for h in range(HPC):
                nc.scalar.activation(
                    out=s4[:, h, :], in_=zq[:, h, 0:NB], func=SILU,
                )
                # w_stop, weight-stationary: out[o, w] in PSUM bank h of sp
                nc.tensor.matmul(
                    sp[:, h, 0:NB], lhsT=wst_sb, rhs=s4[:, h, :],
                    start=True, stop=True,
                )
                nc.vector.tensor_copy(out=out_sb[:, h, :], in_=sp[:, h, 0:NB])
                nc.gpsimd.dma_start(out=out_d[:, h, :], in_=out_sb[:, h, :])

    if not nc.is_finalized():
        nc.finalize()
    return nc


def _prep_host_inputs(k, pe, w_down, w_stop):
    import ml_dtypes

    bf16 = ml_dtypes.bfloat16
    k = np.asarray(k, dtype=np.float32)
    pe = np.asarray(pe, dtype=np.float32)
    w_down = np.asarray(w_down, dtype=np.float32)
    w_stop = np.asarray(w_stop, dtype=np.float32)

    # k5[core, e, d, h, parity, c] = k[head 4*core+h, row 16c + 2e + parity, d]
    k4 = k.reshape(NCORES, HPC, 512, 8, 2, 128).astype(bf16)
    k5 = np.ascontiguousarray(k4.transpose(0, 3, 5, 1, 4, 2))
    # wdt[i, half, d_in, o] = w_down[i][o, 128*half + d_in]
    wdt = np.ascontiguousarray(
        w_down.transpose(0, 2, 1).reshape(5, 2, 128, 128)
    ).astype(bf16)
    # pe0[o, j] = sum_i w_down[0][o, i] * concat(pe[2j], pe[2j+1])[i]
    pe_pairs = pe.reshape(16, 256).astype(np.float64)
    pe0 = (w_down[0].astype(np.float64) @ pe_pairs.T).astype(np.float32)
    wst = np.ascontiguousarray(w_stop.T).astype(bf16)
    return k5, wdt, pe0, wst


def run(k, pe, w_down, w_stop, trace=False, trace_kwargs=None):
    from concourse.bass_utils import run_bass_kernel_spmd

    k5, wdt, pe0, wst = _prep_host_inputs(k, pe, w_down, w_stop)

    if "nc" not in _BASS_CACHE:
        _BASS_CACHE["nc"] = _build_bass()
    nc = _BASS_CACHE["nc"]

    in_maps = [
        {
            "k5": np.ascontiguousarray(k5[c]),
            "wdt": wdt,
            "pe0": pe0,
            "wst": wst,
        }
        for c in range(NCORES)
    ]
    res = run_bass_kernel_spmd(
        nc, in_maps, core_ids=list(range(NCORES)), trace=trace,
        **(trace_kwargs or {}),
    )
    out = np.empty((BH, NW, D), dtype=np.float32)
    for c in range(NCORES):
        r = np.asarray(res.results[c]["out_d"], dtype=np.float32)  # [o, h, w]
        for hh in range(HPC):
            row = HPC * c + hh
            out[row, 0, :] = 0.0
            out[row, 1:, :] = r[:, hh, :].T
    out = out.reshape(B, H, NW, D)
    return out, res


def kernel(k, pe, w_down, w_stop):
    out, _ = run(k, pe, w_down, w_stop, trace=False)
    return out


# revision 4
# speedup vs baseline: 1.0465x; 1.0465x over previous
"""Trainium2 Bass kernel for nn_Compressor (sparse_attention, hierarchical window MLP).

Reference computation (per batch b, head h):
  windows w=0..510 over k[b,h] (S=8192, D=128), window length 32, stride 16
  x[w, l, :] = k[16w+l, :] + pe[l, :]
  5 stages of pairwise-merge MLP: x <- silu(x.reshape(-1, 256) @ w_down[i].T)
  out[w+1] = x @ w_stop.T   ; out[0] = 0 (prepended zero window)

Sharding: head-parallel across 8 cores (B*H = 32 -> 4 heads/core), weights
replicated, no cross-device comms.

Stage-0 sharing: adjacent row pairs (2t, 2t+1) are shared by two windows in
the same even/odd role, so Z[:, t] = W0_even @ kT[:, 2t] + W0_odd @ kT[:, 2t+1]
is computed once per pair; the window-position-dependent pe enters through the
ScalarE activation bias: s0[:, j, w] = silu(Z[:, 8w+j'] + W0 @ pe_pair_j).

Schedule: the core's 4 heads run in lockstep so each ScalarE activation covers
4*511 = 2044 columns (ACT is the bottleneck: 1 cycle/element at 1.2 GHz plus
~290 ns per instruction).  PSUM is two 4-bank quads; ALL work units (stage-0
Z, merge stages, stop) alternate between the quads in emission order, so every
unit's matmuls depend only on the activation two units back - ACT stays
saturated and PE gaps stay short (keeps the HAM clock gate at 2.4 GHz).
k is pre-transposed on the host into per-e-plane chunks so device DMAs are
plain contiguous transfers, streamed through a 3-buffer pool.
"""

import numpy as np

B, H, S, D = 2, 16, 8192, 128
BH = B * H
NCORES = 8
HPC = BH // NCORES  # heads per core = 4
NB = (S - 32) // 16 + 1  # 511 sliding windows
NW = NB + 1  # 512 output rows per head (incl. zero window)

_BASS_CACHE = {}


def _build_bass():
    import concourse.bacc as bacc
    import concourse.mybir as mybir
    import concourse.tile as tile

    f32 = mybir.dt.float32
    bf16 = mybir.dt.bfloat16
    SILU = mybir.ActivationFunctionType.Silu

    nc = bacc.Bacc()
    # k5[e, d, h, parity, c] = k[head h, row 16c + 2e + parity, d]  (bf16)
    k5 = nc.dram_tensor("k5", [8, 128, HPC, 2, 512], bf16, kind="ExternalInput")
    wdt = nc.dram_tensor("wdt", [5, 2, 128, 128], bf16, kind="ExternalInput")
    pe0 = nc.dram_tensor("pe0", [128, 16], f32, kind="ExternalInput")
    wst = nc.dram_tensor("wst", [128, 128], bf16, kind="ExternalInput")
    # out_d[o, h, w] = out[head h, window w+1, feature o]  (bf16, host transposes)
    out_d = nc.dram_tensor("out_d", [128, HPC, NB], bf16, kind="ExternalOutput")

    with tile.TileContext(nc) as tc:
        with (
            tc.tile_pool(name="consts", bufs=1) as consts,
            tc.tile_pool(name="kqp", bufs=3) as kqp,
            tc.tile_pool(name="sbp", bufs=1) as sbp,
            tc.tile_pool(name="psA", bufs=1, space="PSUM") as psA,
            tc.tile_pool(name="psB", bufs=1, space="PSUM") as psB,
        ):
            wd_sb = consts.tile([128, 5, 2, 128], bf16, name="wd_sb")
            nc.gpsimd.dma_start(out=wd_sb, in_=wdt.rearrange("i h k o -> k i h o"))
            pe0_sb = consts.tile([128, 16], f32, name="pe0_sb")
            nc.gpsimd.dma_start(out=pe0_sb, in_=pe0[:])
            wst_sb = consts.tile([128, 128], bf16, name="wst_sb")
            nc.gpsimd.dma_start(out=wst_sb, in_=wst[:])
            probe = consts.tile([128, 4], bf16, name="probe")

            # two 4-bank PSUM quads (bank = one head), alternated across units
            regions = [
                psA.tile([128, HPC, 512], f32, name="zq"),
                psB.tile([128, HPC, 512], f32, name="sp"),
            ]

            # SBUF stage outputs, all [128, planes, heads, windows] bf16
            s0 = sbp.tile([128, 16, HPC, NB], bf16, name="s0")
            s1 = sbp.tile([128, 8, HPC, NB], bf16, name="s1")
            s2 = sbp.tile([128, 4, HPC, NB], bf16, name="s2")
            s3 = sbp.tile([128, 2, HPC, NB], bf16, name="s3")
            s4 = sbp.tile([128, HPC, NB], bf16, name="s4")
            out_sb = sbp.tile([128, HPC, NB], bf16, name="out_sb")

            kts = {}

            def dma_e(e, eng, split=False):
                kq = kqp.tile([128, HPC, 2, 512], bf16, name="kq")
                if split:
                    eng.dma_start(out=kq[:, 0:2], in_=k5[e, :, 0:2])
                    eng.dma_start(out=kq[:, 2:4], in_=k5[e, :, 2:4])
                else:
                    eng.dma_start(out=kq, in_=k5[e])
                kts[e] = kq

            # stagger the first three chunks: e0 split in halves on sync,
            # e1/e2 on the scalar queue serialized behind probe copies so
            # their transfers don't steal SDMA bandwidth from chunk 0
            # (ACT is idle until the first chunk lands anyway)
            dma_e(0, nc.sync, split=True)
            nc.scalar.copy(out=probe, in_=kts[0][:, 3, 1, 0:4])
            dma_e(1, nc.scalar)
            nc.scalar.copy(out=probe, in_=kts[1][:, 3, 1, 0:4])
            dma_e(2, nc.scalar)

            # HAM warmup: chunky matmuls into region B (its first real user,
            # Z1, overwrites with start=True) so PE is at 2.4 GHz when data
            # arrives
            for i in range(16):
                nc.tensor.matmul(
                    regions[1][:, i % 4, 0:256], lhsT=wd_sb[:, 0, 0, :],
                    rhs=wd_sb[:, i % 5, :, :], start=True, stop=True,
                )

            def mm_group(reg, st, h0, rhs_of, n=512):
                """[LDW We, MM h0, MM h0+1, LDW Wo, MM h0, MM h0+1] accumulation."""
                for par in range(2):
                    for h in (h0, h0 + 1):
                        nc.tensor.matmul(
                            reg[:, h, 0:n], lhsT=wd_sb[:, st, par, :],
                            rhs=rhs_of(h, par),
                            start=(par == 0), stop=(par == 1),
                        )

            def z_unit(e, reg):
                kq = kts.pop(e)
                for h0 in (0, 2):
                    mm_group(reg, 0, h0, lambda h, par: kq[:, h, par, :])
                nc.scalar.activation(
                    out=s0[:, e, :, :], in_=reg[:, :, 0:NB], func=SILU,
                    bias=pe0_sb[:, e : e + 1], scale=1.0,
                )
                nc.scalar.activation(
                    out=s0[:, e + 8, :, :], in_=reg[:, :, 1 : NB + 1], func=SILU,
                    bias=pe0_sb[:, e + 8 : e + 9], scale=1.0,
                )

            def stage_unit(st, p, prev, cur, reg):
                for h0 in (0, 2):
                    mm_group(reg, st, h0, lambda h, par: prev[:, 2 * p + par, h, :],
                             n=NB)
                nc.scalar.activation(
                    out=cur[:, p, :, :], in_=reg[:, :, 0:NB], func=SILU,
                )

            # unit list: every unit's data+region deps are >=2 units back
            units = [("z", 0), ("z", 1), ("z", 2), ("s", 1, 0), ("z", 3),
                     ("s", 1, 4), ("z", 4), ("s", 1, 1), ("z", 5), ("s", 1, 5),
                     ("z", 6), ("s", 1, 2), ("z", 7), ("s", 1, 6), ("s", 1, 3),
                     ("s", 1, 7),
                     ("s", 2, 0), ("s", 2, 1), ("s", 2, 2), ("s", 2, 3),
                     ("s", 3, 0), ("s", 3, 1)]
            sin = {1: (s0, s1), 2: (s1, s2), 3: (s2, s3)}
            for n, u in enumerate(units):
                reg = regions[n % 2]
                if u[0] == "z":
                    e = u[1]
                    z_unit(e, reg)
                    if e + 3 < 8:
                        dma_e(e + 3, nc.sync)
                else:
                    _, st, p = u
                    prev, cur = sin[st]
                    stage_unit(st, p, prev, cur, reg)

            # stage 4 (single plane) into the next region, half-split ACTs so
            # the stop pipeline drains per head-pair
            n = len(units)
            reg4 = regions[n % 2]
            for h0 in (0, 2):
                mm_group(reg4, 4, h0, lambda h, par: s3[:, par, h, :], n=NB)
            regS = regions[(n + 1) % 2]
            for h0 in (0, 2):
                nc.scalar.activation(
                    out=s4[:, h0 : h0 + 2, :], in_=reg4[:, h0 : h0 + 2, 0:NB],
                    func=SILU,
                )
                # w_stop, weight-stationary: out[o, w] in PSUM bank h of regS
                for h in (h0, h0 + 1):
                    nc.tensor.matmul(
                        regS[:, h, 0:NB], lhsT=wst_sb, rhs=s4[:, h, :],
                        start=True, stop=True,
                    )
                # copies split across ScalarE and VectorE to halve the drain
                nc.scalar.copy(out=out_sb[:, h0, :], in_=regS[:, h0, 0:NB])
                nc.vector.tensor_copy(
                    out=out_sb[:, h0 + 1, :], in_=regS[:, h0 + 1, 0:NB]
                )
                nc.gpsimd.dma_start(
                    out=out_d[:, h0 : h0 + 2, :], in_=out_sb[:, h0 : h0 + 2, :]
                )

    if not nc.is_finalized():
        nc.finalize()
    return nc


def _prep_host_inputs(k, pe, w_down, w_stop):
    import ml_dtypes

    bf16 = ml_dtypes.bfloat16
    k = np.asarray(k, dtype=np.float32)
    pe = np.asarray(pe, dtype=np.float32)
    w_down = np.asarray(w_down, dtype=np.float32)
    w_stop = np.asarray(w_stop, dtype=np.float32)

    # k5[core, e, d, h, parity, c] = k[head 4*core+h, row 16c + 2e + parity, d]
    k4 = k.reshape(NCORES, HPC, 512, 8, 2, 128).astype(bf16)
    k5 = np.ascontiguousarray(k4.transpose(0, 3, 5, 1, 4, 2))
    # wdt[i, half, d_in, o] = w_down[i][o, 128*half + d_in]
    wdt = np.ascontiguousarray(
        w_down.transpose(0, 2, 1).reshape(5, 2, 128, 128)
    ).astype(bf16)
    # pe0[o, j] = sum_i w_down[0][o, i] * concat(pe[2j], pe[2j+1])[i]
    pe_pairs = pe.reshape(16, 256).astype(np.float64)
    pe0 = (w_down[0].astype(np.float64) @ pe_pairs.T).astype(np.float32)
    wst = np.ascontiguousarray(w_stop.T).astype(bf16)
    return k5, wdt, pe0, wst


def run(k, pe, w_down, w_stop, trace=False, trace_kwargs=None):
    from concourse.bass_utils import run_bass_kernel_spmd

    k5, wdt, pe0, wst = _prep_host_inputs(k, pe, w_down, w_stop)

    if "nc" not in _BASS_CACHE:
        _BASS_CACHE["nc"] = _build_bass()
    nc = _BASS_CACHE["nc"]

    in_maps = [
        {
            "k5": np.ascontiguousarray(k5[c]),
            "wdt": wdt,
            "pe0": pe0,
            "wst": wst,
        }
        for c in range(NCORES)
    ]
    res = run_bass_kernel_spmd(
        nc, in_maps, core_ids=list(range(NCORES)), trace=trace,
        **(trace_kwargs or {}),
    )
    out = np.empty((BH, NW, D), dtype=np.float32)
    for c in range(NCORES):
        r = np.asarray(res.results[c]["out_d"], dtype=np.float32)  # [o, h, w]
        for hh in range(HPC):
            row = HPC * c + hh
            out[row, 0, :] = 0.0
            out[row, 1:, :] = r[:, hh, :].T
    out = out.reshape(B, H, NW, D)
    return out, res


def kernel(k, pe, w_down, w_stop):
    out, _ = run(k, pe, w_down, w_stop, trace=False)
    return out
